# revision 5
# baseline (speedup 1.0000x reference)
"""AnticipatorRNN Trainium2 kernel: conv encoder + 2-layer LSTM + dense head.

Data-parallel over batch B=16 across 8 NeuronCores (2 sequences/core).
All matmuls in bf16 (f32 psum accumulate).

Per-dispatch input bytes are minimized (they dominate wall time on this
transport): frames ship as space-to-depth bf16 (conv1 im2col taps are
built on device), and the big weight matrices ship sharded 1/8-per-core
and are AllGathered on device.
"""

import os
import numpy as np
import ml_dtypes

import concourse.bass as bass
import concourse.mybir as mybir
from concourse import bacc
import concourse.tile as tile

F32 = mybir.dt.float32
BF16 = mybir.dt.bfloat16
AF = mybir.ActivationFunctionType

B, T, H, W, C = 16, 64, 64, 64, 3
NCORES = 8
BP = B // NCORES          # sequences per core = 2
F = BP * T                # frames per core = 128
O1, O2, O3, O4 = 31, 14, 6, 2
N1, N2, N3, N4 = O1 * O1, O2 * O2, O3 * O3, O4 * O4   # 961, 196, 36, 4

# packed big-weight buffer column offsets (bf16, all rows=128)
OX1, OH1, OL2, O4W, OD1, O3W = 0, 16384, 24576, 30720, 34816, 35072
CPACK = 36096

_CACHE = {}
TAPS = [(a, b) for a in range(4) for b in range(4)]


def _build_graph(phases=None):
    if phases is None:
        phases = set((os.environ.get("KPHASES")
                      or "conv12,conv3,conv4,xw1,lstm,head").split(","))
    nc = bacc.Bacc(None, target_bir_lowering=False, num_devices=NCORES)

    noag = os.environ.get("KNOAG", "0") == "1"
    zdp = nc.declare_dram_parameter("zd", [12, F, 32, 32], BF16, isOutput=False)
    actp = nc.declare_dram_parameter("actT", [2, F], BF16, isOutput=False)
    if noag:
        wbigp = nc.declare_dram_parameter("wbig", [128, CPACK], BF16,
                                          isOutput=False)
    else:
        wshp = nc.declare_dram_parameter("wsh", [16, CPACK], BF16,
                                         isOutput=False)
    w1p = nc.declare_dram_parameter("w1b", [24, 2, 32], BF16, isOutput=False)
    b1p = nc.declare_dram_parameter("b1p", [128, 1], F32, isOutput=False)
    w2p = nc.declare_dram_parameter("w2p", [32, 16, 64], BF16, isOutput=False)
    b2p = nc.declare_dram_parameter("b2p", [128, 1], F32, isOutput=False)
    b3p = nc.declare_dram_parameter("b3p", [128, 1], F32, isOutput=False)
    b4p = nc.declare_dram_parameter("b4p", [128, 2], F32, isOutput=False)
    wactp = nc.declare_dram_parameter("wactp", [2, 2048], BF16, isOutput=False)
    bl1p = nc.declare_dram_parameter("bl1p", [128, 16], F32, isOutput=False)
    bl2p = nc.declare_dram_parameter("bl2p", [128, 16], F32, isOutput=False)
    bd1p = nc.declare_dram_parameter("bd1p", [128, 1], F32, isOutput=False)
    wd2p = nc.declare_dram_parameter("wd2p", [128, 1], BF16, isOutput=False)
    bd2p = nc.declare_dram_parameter("bd2p", [1, 1], F32, isOutput=False)
    outp = nc.declare_dram_parameter("out", [1, F], F32, isOutput=True)

    with tile.TileContext(nc) as tc:
        with (
            tc.tile_pool(name="dram", bufs=1, space="DRAM") as dpool,
            tc.tile_pool(name="const", bufs=1) as const,
            tc.tile_pool(name="ic", bufs=4) as icpool,
            tc.tile_pool(name="x2", bufs=4) as x2pool,
            tc.tile_pool(name="big", bufs=1) as big,
            tc.tile_pool(name="ps", bufs=2, space="PSUM") as pspool,
            tc.tile_pool(name="pss", bufs=2, space="PSUM") as psspool,
            tc.tile_pool(name="st", bufs=3) as st,
            tc.tile_pool(name="sc", bufs=4) as sc,
        ):
            # ---- weight AllGather: shard -> bounce -> full pack ----
            if noag:
                wbig = wbigp
            else:
                wsh_int = dpool.tile([16, CPACK], BF16)
                wbig = dpool.tile([128, CPACK], BF16)
                nc.gpsimd.dma_start(wsh_int[:], wshp[:, :])
                nc.gpsimd.collective_compute(
                    "AllGather",
                    mybir.AluOpType.bypass,
                    replica_groups=[list(range(NCORES))],
                    ins=[wsh_int.opt()],
                    outs=[wbig.opt()],
                )

            # ---- load weights/constants into SBUF ----
            w1s = const.tile([24, 2, 32], BF16)
            nc.sync.dma_start(out=w1s[:, :, :], in_=w1p[:, :, :])
            b1s = const.tile([128, 1], F32)
            nc.sync.dma_start(out=b1s[:, :], in_=b1p[:, :])
            w2s = const.tile([128, 16, 64], BF16)
            for j in range(4):
                nc.sync.dma_start(out=w2s[32 * j:32 * j + 32, :, :], in_=w2p[:, :, :])
            b2s = const.tile([128, 1], F32)
            nc.sync.dma_start(out=b2s[:, :], in_=b2p[:, :])
            w3s = const.tile([128, 16, 128], BF16)
            w3f = w3s.rearrange("p a b -> p (a b)")
            for h in range(2):
                nc.sync.dma_start(out=w3f[64 * h:64 * h + 64, 0:1024],
                                  in_=wbig[0:64, O3W:O3W + 1024])
                nc.sync.dma_start(out=w3f[64 * h:64 * h + 64, 1024:2048],
                                  in_=wbig[64:128, O3W:O3W + 1024])
            b3s = const.tile([128, 1], F32)
            nc.sync.dma_start(out=b3s[:, :], in_=b3p[:, :])
            w4s = const.tile([128, 16, 256], BF16)
            nc.sync.dma_start(out=w4s.rearrange("p a b -> p (a b)")[:, :],
                              in_=wbig[:, O4W:O4W + 4096])
            b4s = const.tile([128, 2], F32)
            nc.sync.dma_start(out=b4s[:, :], in_=b4p[:, :])
            wx1s = const.tile([128, 8, 2048], BF16)
            nc.sync.dma_start(out=wx1s.rearrange("p a b -> p (a b)")[:, :],
                              in_=wbig[:, OX1:OX1 + 16384])
            wacts = const.tile([2, 2048], BF16)
            nc.sync.dma_start(out=wacts[:, :], in_=wactp[:, :])
            wh1s = const.tile([128, 4, 2048], BF16)
            nc.sync.dma_start(out=wh1s.rearrange("p a b -> p (a b)")[:, :],
                              in_=wbig[:, OH1:OH1 + 8192])
            bl1s = const.tile([128, 16], F32)
            nc.sync.dma_start(out=bl1s[:, :], in_=bl1p[:, :])
            wl2s = const.tile([128, 6, 1024], BF16)
            nc.sync.dma_start(out=wl2s.rearrange("p a b -> p (a b)")[:, :],
                              in_=wbig[:, OL2:OL2 + 6144])
            bl2s = const.tile([128, 16], F32)
            nc.sync.dma_start(out=bl2s[:, :], in_=bl2p[:, :])
            wd1s = const.tile([128, 2, 128], BF16)
            nc.sync.dma_start(out=wd1s.rearrange("p a b -> p (a b)")[:, :],
                              in_=wbig[:, OD1:OD1 + 256])
            bd1s = const.tile([128, 1], F32)
            nc.sync.dma_start(out=bd1s[:, :], in_=bd1p[:, :])
            wd2s = const.tile([128, 1], BF16)
            nc.sync.dma_start(out=wd2s[:, :], in_=wd2p[:, :])
            bd2s = const.tile([1, 1], F32)
            nc.sync.dma_start(out=bd2s[:, :], in_=bd2p[:, :])
            acts = const.tile([2, F], BF16)
            nc.sync.dma_start(out=acts[:, :], in_=actp[:, :])

            # persistent activations
            x3s = big.tile([128, 64, N2], BF16)     # [(half,ch64), pair, 196]
            x4s = big.tile([128, F, N3], BF16)      # [ch128, f, 36]
            encs = big.tile([128, 8, F], BF16)      # [row, ktile, (b,t)]
            xw1s = big.tile([128, 16, F], F32)      # [gate-row, mtile, (b,t)]
            h2hist = big.tile([128, 2, F], BF16)    # [row, ktile, (b,t)]

            reps = int(os.environ.get("KREPS", "1"))
            full = {"conv12", "conv3", "conv4", "xw1", "lstm", "head"} <= phases
            if not full:
                nc.any.memset(x3s[:, :, :], 0.0)
                nc.any.memset(x4s[:, :, :], 0.0)
                nc.any.memset(encs[:, :, :], 0.0)
                nc.any.memset(xw1s[:, :, :], 0.0)
                nc.any.memset(h2hist[:, :, :], 0.0)

            for _rep in range(reps):
                # ---- conv1 + conv2, per 4-frame group ----
                if "conv12" in phases:
                    for g in range(F // 4):
                        ps1 = pspool.tile([128, 2, 512], F32, tag="ps")
                        for j in range(4):
                            f = 4 * g + j
                            ict = icpool.tile([24, 32, 32], BF16, tag="ic")
                            for a in range(2):
                                nc.sync.dma_start(
                                    out=ict[12 * a:12 * a + 12, 0:31, :],
                                    in_=zdp[:, f, a:a + 31, :])
                            for b in range(2):
                                for r0, nr, n0 in ((0, 16, 0), (16, 15, 0)):
                                    bank = 0 if r0 == 0 else 1
                                    nc.tensor.matmul(
                                        ps1[32 * j:32 * j + 32, bank,
                                            0:nr * O1],
                                        w1s[:, b, :],
                                        ict[:, r0:r0 + nr, b:b + O1],
                                        start=(b == 0), stop=(b == 1),
                                        tile_position=(0, 32 * j),
                                    )
                        x2t = x2pool.tile([128, O1, O1], BF16, tag="x2")
                        x2f = x2t.rearrange("p a b -> p (a b)")
                        nc.scalar.activation(x2f[:, 0:496], ps1[:, 0, 0:496],
                                             AF.Relu, bias=b1s[:, :])
                        nc.scalar.activation(x2f[:, 496:961], ps1[:, 1, 0:465],
                                             AF.Relu, bias=b1s[:, :])

                        ps2 = pspool.tile([128, 2, 512], F32, tag="ps")
                        for it, (kh, kw) in enumerate(TAPS):
                            for j in range(4):
                                half = j // 2
                                nc.tensor.matmul(
                                    ps2[64 * half:64 * half + 64, j % 2, 0:N2],
                                    w2s[32 * j:32 * j + 32, it, :],
                                    x2t[32 * j:32 * j + 32,
                                        kh:kh + 27:2, kw:kw + 27:2],
                                    start=(it == 0), stop=(it == 15),
                                    tile_position=(32 * j, 64 * half),
                                )
                        for k in range(2):
                            nc.scalar.activation(x3s[:, 2 * g + k, :],
                                                 ps2[:, k, 0:N2],
                                                 AF.Relu, bias=b2s[:, :])

                # ---- conv3: 16 taps K=64, pairs batched 8 per matmul ----
                if "conv3" in phases:
                    x3v = x3s.rearrange("p n (r s) -> p n r s", r=O2)
                    x4q = x4s.rearrange("p (q i l) x -> p q i l x", q=8, i=4, l=4)
                    for q in range(8):
                        ps3 = pspool.tile([128, 2, 512], F32, tag="ps")
                        for it, (kh, kw) in enumerate(TAPS):
                            for h in range(2):
                                nc.tensor.matmul(
                                    ps3[:, h, 0:8 * N3],
                                    w3s[64 * h:64 * h + 64, it, :],
                                    x3v[64 * h:64 * h + 64, 8 * q:8 * q + 8,
                                        kh:kh + 11:2, kw:kw + 11:2],
                                    start=(it == 0), stop=(it == 15),
                                    tile_position=(64 * h, 0),
                                )
                        for h in range(2):
                            nc.scalar.activation(
                                x4q[:, q, :, 2 * h:2 * h + 2, :],
                                ps3[:, h, 0:8 * N3].rearrange(
                                    "p (n x) -> p n x", x=N3),
                                AF.Relu, bias=b3s[:, :])

                # ---- conv4: 16 taps K=128, all frames batched ----
                if "conv4" in phases:
                    x4v = x4s.rearrange("p f (r s) -> p f r s", r=O3)
                    ps4 = pspool.tile([128, 2, 512], F32, tag="ps")
                    for it, (kh, kw) in enumerate(TAPS):
                        for mh in range(2):
                            nc.tensor.matmul(
                                ps4[:, mh, :],
                                w4s[:, it, 128 * mh:128 * mh + 128],
                                x4v[:, :, kh:kh + 3:2, kw:kw + 3:2],
                                start=(it == 0), stop=(it == 15),
                            )
                    ps4v = ps4.rearrange("p m (f x) -> p m f x", x=N4)
                    for p in range(4):
                        for mh in range(2):
                            nc.scalar.activation(encs[:, 2 * p + mh, :],
                                                 ps4v[:, mh, :, p],
                                                 AF.Relu, bias=b4s[:, mh:mh + 1])

                # ---- x-projection for LSTM layer 1, batched over (b,t) ----
                if "xw1" in phases:
                    for m in range(16):
                        psx = psspool.tile([128, F], F32, tag="pss")
                        for kt in range(8):
                            nc.tensor.matmul(psx[:, :],
                                             wx1s[:, kt, 128 * m:128 * m + 128],
                                             encs[:, kt, :],
                                             start=(kt == 0), stop=False)
                        nc.tensor.matmul(psx[:, :],
                                         wacts[:, 128 * m:128 * m + 128],
                                         acts[:, :],
                                         start=False, stop=True)
                        nc.scalar.activation(xw1s[:, m, :], psx[:, :],
                                             AF.Identity, bias=bl1s[:, m:m + 1])

                # ---- LSTM recurrence, 64 steps, batch=2 per core ----
                if "lstm" in phases:
                    h1 = st.tile([128, 8], BF16, tag="h1z")
                    c1 = st.tile([128, 8], F32, tag="c1z")
                    h2 = st.tile([128, 4], BF16, tag="h2z")
                    c2 = st.tile([128, 4], F32, tag="c2z")
                    nc.any.memset(h1[:, :], 0.0)
                    nc.any.memset(c1[:, :], 0.0)
                    nc.any.memset(h2[:, :], 0.0)
                    nc.any.memset(c2[:, :], 0.0)
                    xw1v = xw1s.rearrange("p m (b t) -> p m b t", b=BP)
                    hv = h2hist.rearrange("p k (b t) -> p k b t", b=BP)

                    for t in range(T):
                        pg1 = psspool.tile([128, 32], F32, tag="pss")
                        for m in range(16):
                            for kt in range(4):
                                nc.tensor.matmul(pg1[:, 2 * m:2 * m + 2],
                                                 wh1s[:, kt, 128 * m:128 * m + 128],
                                                 h1[:, 2 * kt:2 * kt + 2],
                                                 start=(kt == 0), stop=(kt == 3))
                        g1 = sc.tile([128, 32], F32, tag="g1")
                        nc.vector.tensor_add(g1[:, :], pg1[:, :], xw1v[:, :, :, t])
                        nc.scalar.activation(g1[:, 0:8], g1[:, 0:8], AF.Sigmoid)
                        nc.scalar.activation(g1[:, 8:16], g1[:, 8:16], AF.Tanh)
                        nc.scalar.activation(g1[:, 16:32], g1[:, 16:32], AF.Sigmoid)
                        t1 = sc.tile([128, 8], F32, tag="t1")
                        nc.vector.tensor_mul(t1[:, :], g1[:, 16:24], c1[:, :])
                        t2 = sc.tile([128, 8], F32, tag="t2")
                        nc.vector.tensor_mul(t2[:, :], g1[:, 0:8], g1[:, 8:16])
                        c1 = st.tile([128, 8], F32, tag="c1")
                        nc.vector.tensor_add(c1[:, :], t1[:, :], t2[:, :])
                        th1 = sc.tile([128, 8], F32, tag="th1")
                        nc.scalar.activation(th1[:, :], c1[:, :], AF.Tanh)
                        h1 = st.tile([128, 8], BF16, tag="h1")
                        nc.vector.tensor_mul(h1[:, :], g1[:, 24:32], th1[:, :])

                        pg2 = psspool.tile([128, 16], F32, tag="pss")
                        for m in range(8):
                            for kt in range(6):
                                rhs = (h1[:, 2 * kt:2 * kt + 2] if kt < 4
                                       else h2[:, 2 * (kt - 4):2 * (kt - 4) + 2])
                                nc.tensor.matmul(pg2[:, 2 * m:2 * m + 2],
                                                 wl2s[:, kt, 128 * m:128 * m + 128],
                                                 rhs,
                                                 start=(kt == 0), stop=(kt == 5))
                        g2 = sc.tile([128, 16], F32, tag="g2")
                        nc.vector.tensor_add(g2[:, :], pg2[:, :], bl2s[:, :])
                        nc.scalar.activation(g2[:, 0:4], g2[:, 0:4], AF.Sigmoid)
                        nc.scalar.activation(g2[:, 4:8], g2[:, 4:8], AF.Tanh)
                        nc.scalar.activation(g2[:, 8:16], g2[:, 8:16], AF.Sigmoid)
                        t3 = sc.tile([128, 4], F32, tag="t3")
                        nc.vector.tensor_mul(t3[:, :], g2[:, 8:12], c2[:, :])
                        t4 = sc.tile([128, 4], F32, tag="t4")
                        nc.vector.tensor_mul(t4[:, :], g2[:, 0:4], g2[:, 4:8])
                        c2 = st.tile([128, 4], F32, tag="c2")
                        nc.vector.tensor_add(c2[:, :], t3[:, :], t4[:, :])
                        th2 = sc.tile([128, 4], F32, tag="th2")
                        nc.scalar.activation(th2[:, :], c2[:, :], AF.Tanh)
                        h2 = st.tile([128, 4], BF16, tag="h2")
                        nc.vector.tensor_mul(h2[:, :], g2[:, 12:16], th2[:, :])
                        nc.vector.tensor_copy(
                            hv[:, :, :, t],
                            h2[:, :].rearrange("p (k b) -> p k b", k=2))

                # ---- dense head, batched over (b,t) ----
                if "head" in phases:
                    pd1 = psspool.tile([128, F], F32, tag="pss")
                    for kt in range(2):
                        nc.tensor.matmul(pd1[:, :], wd1s[:, kt, :],
                                         h2hist[:, kt, :],
                                         start=(kt == 0), stop=(kt == 1))
                    d1t = sc.tile([128, F], BF16, tag="d1t")
                    nc.scalar.activation(d1t[:, :], pd1[:, :], AF.Relu,
                                         bias=bd1s[:, :])
                    pd2 = psspool.tile([1, F], F32, tag="pss")
                    nc.tensor.matmul(pd2[:, :], wd2s[:, :], d1t[:, :],
                                     start=True, stop=True)
                    osb = sc.tile([1, F], F32, tag="osb")
                    nc.scalar.activation(osb[:, :], pd2[:, :], AF.Identity,
                                         bias=bd2s[:, :])
                else:
                    osb = sc.tile([1, F], F32, tag="osb")
                    nc.any.memset(osb[:, :], 0.0)
            nc.sync.dma_start(out=outp[:, :], in_=osb[:, :])

    nc.finalize()
    return nc


def _prep_host(inputs):
    bf = ml_dtypes.bfloat16
    frames = np.ascontiguousarray(inputs["frames"], dtype=np.float32)
    actions = np.ascontiguousarray(inputs["actions"], dtype=np.float32)

    shared = {}
    w1r = np.asarray(inputs["w1"], np.float32).reshape(2, 2, 2, 2, 3, 32)
    # rows (a, di, dj, c), second dim b: matches device ict2 layout
    shared["w1b"] = np.ascontiguousarray(
        w1r.transpose(0, 1, 3, 4, 2, 5).reshape(24, 2, 32)).astype(bf)
    shared["b1p"] = np.tile(np.asarray(inputs["b1"], np.float32), 4)[:, None].copy()
    shared["w2p"] = np.ascontiguousarray(
        np.asarray(inputs["w2"], np.float32).reshape(16, 32, 64).transpose(1, 0, 2)
    ).astype(bf)
    shared["b2p"] = np.tile(np.asarray(inputs["b2"], np.float32), 2)[:, None].copy()
    shared["b3p"] = np.asarray(inputs["b3"], np.float32)[:, None].copy()
    shared["b4p"] = np.ascontiguousarray(
        np.asarray(inputs["b4"], np.float32).reshape(2, 128).T)
    kl1 = np.asarray(inputs["k_l1"], np.float32)
    shared["wactp"] = np.ascontiguousarray(kl1[1024:1026]).astype(bf)
    bl1 = np.asarray(inputs["b_l1"], np.float32).copy()
    bl1[1024:1536] += 1.0          # fold forget bias
    shared["bl1p"] = np.ascontiguousarray(bl1.reshape(16, 128).T)
    bl2 = np.asarray(inputs["b_l2"], np.float32).copy()
    bl2[512:768] += 1.0
    shared["bl2p"] = np.ascontiguousarray(
        np.repeat(bl2.reshape(8, 128).T, 2, axis=1))
    shared["bd1p"] = np.asarray(inputs["b_d1"], np.float32)[:, None].copy()
    shared["wd2p"] = np.asarray(inputs["w_d2"], np.float32).astype(bf).copy()
    shared["bd2p"] = np.asarray(inputs["b_d2"], np.float32).reshape(1, 1).copy()

    # ---- packed big-weight buffer [128, CPACK], sharded 16 rows/core ----
    wx1 = kl1[0:1024].reshape(8, 128, 2048).transpose(1, 0, 2).reshape(128, 16384)
    wh1 = kl1[1026:1538].reshape(4, 128, 2048).transpose(1, 0, 2).reshape(128, 8192)
    kl2 = np.asarray(inputs["k_l2"], np.float32)
    wl2 = kl2.reshape(6, 128, 1024).transpose(1, 0, 2).reshape(128, 6144)
    w4 = np.asarray(inputs["w4"], np.float32).reshape(16, 128, 256)
    w4 = w4.transpose(1, 0, 2).reshape(128, 4096)
    wd1 = np.asarray(inputs["w_d1"], np.float32).reshape(2, 128, 128)
    wd1 = wd1.transpose(1, 0, 2).reshape(128, 256)
    w3 = np.asarray(inputs["w3"], np.float32).reshape(16, 64, 128)
    w3 = w3.transpose(1, 0, 2).reshape(64, 2048)
    w3pack = np.concatenate([w3[:, 0:1024], w3[:, 1024:2048]], axis=0)
    wbig = np.concatenate([wx1, wh1, wl2, w4, wd1, w3pack], axis=1).astype(bf)
    assert wbig.shape == (128, CPACK)

    in_maps = []
    for c in range(NCORES):
        fr = frames[BP * c:BP * c + BP].reshape(F, H, W, C)
        z = fr.reshape(F, 32, 2, 32, 2, 3).transpose(2, 4, 5, 0, 1, 3)
        zd = np.ascontiguousarray(z.reshape(12, F, 32, 32)).astype(bf)
        actT = np.ascontiguousarray(
            actions[BP * c:BP * c + BP].reshape(F, 2).T).astype(bf)
        m = {"zd": zd, "actT": actT,
             "wsh": np.ascontiguousarray(wbig[16 * c:16 * c + 16]),
             "wbig": wbig}
        m.update(shared)
        in_maps.append(m)
    return in_maps


def kernel(**inputs):
    from concourse.bass_utils import run_bass_kernel_spmd
    if "nc" not in _CACHE:
        _CACHE["nc"] = _build_graph()
    nc = _CACHE["nc"]
    in_maps = _prep_host(inputs)
    res = run_bass_kernel_spmd(nc, in_maps, core_ids=list(range(NCORES)),
                               trace=False)
    outs = [res.results[c]["out"].reshape(BP, T, 1) for c in range(NCORES)]
    return np.concatenate(outs, axis=0)


# revision 11
# speedup vs baseline: 1.0523x; 1.0523x over previous
"""AnticipatorRNN Trainium2 kernel: conv encoder + 2-layer LSTM + dense head.

Data-parallel over batch B=16 across 8 NeuronCores (2 sequences/core).
All matmuls in bf16 (f32 psum accumulate).

Per-dispatch input bytes are minimized (they dominate wall time on this
transport): frames ship as space-to-depth bf16 (conv1 im2col taps are
built on device), and the big weight matrices ship sharded 1/8-per-core
and are AllGathered on device.
"""

import os
import numpy as np
import ml_dtypes

import concourse.bass as bass
import concourse.mybir as mybir
from concourse import bacc
import concourse.tile as tile

F32 = mybir.dt.float32
BF16 = mybir.dt.bfloat16
AF = mybir.ActivationFunctionType

B, T, H, W, C = 16, 64, 64, 64, 3
NCORES = 8
BP = B // NCORES          # sequences per core = 2
F = BP * T                # frames per core = 128
O1, O2, O3, O4 = 31, 14, 6, 2
N1, N2, N3, N4 = O1 * O1, O2 * O2, O3 * O3, O4 * O4   # 961, 196, 36, 4

# packed big-weight buffer column offsets (bf16, all rows=128)
OX1, OH1, OL2, O4W, OD1, O3W = 0, 16384, 24576, 30720, 34816, 35072
CPACK = 36096

_CACHE = {}
TAPS = [(a, b) for a in range(4) for b in range(4)]


def _build_graph(phases=None):
    if phases is None:
        phases = set((os.environ.get("KPHASES")
                      or "conv12,conv3,conv4,xw1,lstm,head").split(","))
    nc = bacc.Bacc(None, target_bir_lowering=False, num_devices=NCORES)

    noag = os.environ.get("KNOAG", "0") == "1"
    zdp = nc.declare_dram_parameter("zd", [12, F, 32, 32], BF16, isOutput=False)
    actp = nc.declare_dram_parameter("actT", [2, F], BF16, isOutput=False)
    if noag:
        wbigp = nc.declare_dram_parameter("wbig", [128, CPACK], BF16,
                                          isOutput=False)
    else:
        wshp = nc.declare_dram_parameter("wsh", [16, CPACK], BF16,
                                         isOutput=False)
    w1p = nc.declare_dram_parameter("w1b", [24, 2, 32], BF16, isOutput=False)
    b1p = nc.declare_dram_parameter("b1p", [128, 1], F32, isOutput=False)
    w2p = nc.declare_dram_parameter("w2p", [32, 16, 64], BF16, isOutput=False)
    b2p = nc.declare_dram_parameter("b2p", [128, 1], F32, isOutput=False)
    b3p = nc.declare_dram_parameter("b3p", [128, 1], F32, isOutput=False)
    b4p = nc.declare_dram_parameter("b4p", [128, 2], F32, isOutput=False)
    wactp = nc.declare_dram_parameter("wactp", [2, 2048], BF16, isOutput=False)
    bl1p = nc.declare_dram_parameter("bl1p", [128, 16], F32, isOutput=False)
    bl2p = nc.declare_dram_parameter("bl2p", [128, 16], F32, isOutput=False)
    bd1p = nc.declare_dram_parameter("bd1p", [128, 1], F32, isOutput=False)
    wd2p = nc.declare_dram_parameter("wd2p", [128, 1], BF16, isOutput=False)
    bd2p = nc.declare_dram_parameter("bd2p", [1, 1], F32, isOutput=False)
    outp = nc.declare_dram_parameter("out", [1, F], F32, isOutput=True)

    with tile.TileContext(nc) as tc:
        with (
            tc.tile_pool(name="dram", bufs=1, space="DRAM") as dpool,
            tc.tile_pool(name="const", bufs=1) as const,
            tc.tile_pool(name="ic", bufs=4) as icpool,
            tc.tile_pool(name="x2", bufs=4) as x2pool,
            tc.tile_pool(name="big", bufs=1) as big,
            tc.tile_pool(name="ps", bufs=2, space="PSUM") as pspool,
            tc.tile_pool(name="pss", bufs=2, space="PSUM") as psspool,
            tc.tile_pool(name="st", bufs=3) as st,
            tc.tile_pool(name="sc", bufs=4) as sc,
        ):
            # ---- weight AllGather: shard -> bounce -> full pack ----
            if noag:
                wbig = wbigp
            else:
                wsh_int = dpool.tile([16, CPACK], BF16)
                wbig = dpool.tile([128, CPACK], BF16)
                nc.gpsimd.dma_start(wsh_int[:], wshp[:, :])
                nc.gpsimd.collective_compute(
                    "AllGather",
                    mybir.AluOpType.bypass,
                    replica_groups=[list(range(NCORES))],
                    ins=[wsh_int.opt()],
                    outs=[wbig.opt()],
                )

            # ---- load weights/constants into SBUF ----
            w1s = const.tile([24, 2, 32], BF16)
            nc.sync.dma_start(out=w1s[:, :, :], in_=w1p[:, :, :])
            b1s = const.tile([128, 1], F32)
            nc.sync.dma_start(out=b1s[:, :], in_=b1p[:, :])
            w2s = const.tile([128, 16, 64], BF16)
            for j in range(4):
                nc.sync.dma_start(out=w2s[32 * j:32 * j + 32, :, :], in_=w2p[:, :, :])
            b2s = const.tile([128, 1], F32)
            nc.sync.dma_start(out=b2s[:, :], in_=b2p[:, :])
            w3s = const.tile([128, 16, 128], BF16)
            w3f = w3s.rearrange("p a b -> p (a b)")
            for h in range(2):
                nc.sync.dma_start(out=w3f[64 * h:64 * h + 64, 0:1024],
                                  in_=wbig[0:64, O3W:O3W + 1024])
                nc.sync.dma_start(out=w3f[64 * h:64 * h + 64, 1024:2048],
                                  in_=wbig[64:128, O3W:O3W + 1024])
            b3s = const.tile([128, 1], F32)
            nc.sync.dma_start(out=b3s[:, :], in_=b3p[:, :])
            w4s = const.tile([128, 16, 256], BF16)
            nc.sync.dma_start(out=w4s.rearrange("p a b -> p (a b)")[:, :],
                              in_=wbig[:, O4W:O4W + 4096])
            b4s = const.tile([128, 2], F32)
            nc.sync.dma_start(out=b4s[:, :], in_=b4p[:, :])
            wx1s = const.tile([128, 8, 2048], BF16)
            nc.sync.dma_start(out=wx1s.rearrange("p a b -> p (a b)")[:, :],
                              in_=wbig[:, OX1:OX1 + 16384])
            wacts = const.tile([2, 2048], BF16)
            nc.sync.dma_start(out=wacts[:, :], in_=wactp[:, :])
            wh1s = const.tile([128, 4, 2048], BF16)
            nc.sync.dma_start(out=wh1s.rearrange("p a b -> p (a b)")[:, :],
                              in_=wbig[:, OH1:OH1 + 8192])
            bl1s = const.tile([128, 16], F32)
            nc.sync.dma_start(out=bl1s[:, :], in_=bl1p[:, :])
            wl2s = const.tile([128, 6, 1024], BF16)
            nc.sync.dma_start(out=wl2s.rearrange("p a b -> p (a b)")[:, :],
                              in_=wbig[:, OL2:OL2 + 6144])
            bl2s = const.tile([128, 16], F32)
            nc.sync.dma_start(out=bl2s[:, :], in_=bl2p[:, :])
            wd1s = const.tile([128, 2, 128], BF16)
            nc.sync.dma_start(out=wd1s.rearrange("p a b -> p (a b)")[:, :],
                              in_=wbig[:, OD1:OD1 + 256])
            bd1s = const.tile([128, 1], F32)
            nc.sync.dma_start(out=bd1s[:, :], in_=bd1p[:, :])
            wd2s = const.tile([128, 1], BF16)
            nc.sync.dma_start(out=wd2s[:, :], in_=wd2p[:, :])
            bd2s = const.tile([1, 1], F32)
            nc.sync.dma_start(out=bd2s[:, :], in_=bd2p[:, :])
            acts = const.tile([2, F], BF16)
            nc.sync.dma_start(out=acts[:, :], in_=actp[:, :])

            # persistent activations
            x3s = big.tile([128, 64, 2, 2, 7, 7], BF16)  # parity planes
            x4s = big.tile([128, F, N3], BF16)      # [ch128, f, 36]
            encs = big.tile([128, 8, F], BF16)      # [row, ktile, (b,t)]
            xw1s = big.tile([128, 16, F], F32)      # [gate-row, mtile, (b,t)]
            h2hist = big.tile([128, 2, F], BF16)    # [row, ktile, (b,t)]

            reps = int(os.environ.get("KREPS", "1"))
            full = {"conv12", "conv3", "conv4", "xw1", "lstm", "head"} <= phases
            if not full:
                nc.any.memset(x3s[:, :, :, :, :, :], 0.0)
                nc.any.memset(x4s[:, :, :], 0.0)
                nc.any.memset(encs[:, :, :], 0.0)
                nc.any.memset(xw1s[:, :, :], 0.0)
                nc.any.memset(h2hist[:, :, :], 0.0)

            for _rep in range(reps):
                # ---- conv1 + conv2, per 4-frame group ----
                # x2 is stored in parity-plane layout [p, ph, pw, 16, 16] so
                # conv2's stride-2 tap windows become contiguous reads.
                if "conv12" in phases:
                    for g in range(F // 4):
                        ps1 = pspool.tile([128, 2, 512], F32, tag="ps")
                        ps1v = ps1.rearrange("p b (o w) -> p b o w", w=32)
                        for j in range(4):
                            f = 4 * g + j
                            ict = icpool.tile([24, 32, 34], BF16, tag="ic")
                            nc.any.memset(ict[:, :, 32:34], 0.0)
                            for a in range(2):
                                nc.sync.dma_start(
                                    out=ict[12 * a:12 * a + 12, 0:31, 0:32],
                                    in_=zdp[:, f, a:a + 31, :])
                            # psum rows padded to width 32 (col 31 = junk)
                            for b in range(2):
                                for r0, nr, bank in ((0, 16, 0), (16, 15, 1)):
                                    nc.tensor.matmul(
                                        ps1[32 * j:32 * j + 32, bank,
                                            0:nr * 32],
                                        w1s[:, b, :],
                                        ict[:, r0:r0 + nr, b:b + 32],
                                        start=(b == 0), stop=(b == 1),
                                        tile_position=(0, 32 * j),
                                    )
                        x2p = x2pool.tile([128, 2, 2, 16, 16], BF16, tag="x2")
                        x2A = x2p.rearrange("p h w r s -> p r h s w")
                        nc.scalar.activation(x2A[:, 0:8, 0, :, :],
                                             ps1v[:, 0, 0:16:2, :],
                                             AF.Relu, bias=b1s[:, :])
                        nc.scalar.activation(x2A[:, 0:8, 1, :, :],
                                             ps1v[:, 0, 1:16:2, :],
                                             AF.Relu, bias=b1s[:, :])
                        nc.scalar.activation(x2A[:, 8:16, 0, :, :],
                                             ps1v[:, 1, 0:15:2, :],
                                             AF.Relu, bias=b1s[:, :])
                        nc.scalar.activation(x2A[:, 8:15, 1, :, :],
                                             ps1v[:, 1, 1:15:2, :],
                                             AF.Relu, bias=b1s[:, :])

                        ps2 = pspool.tile([128, 2, 512], F32, tag="ps")
                        for it, (kh, kw) in enumerate(TAPS):
                            for j in range(4):
                                half = j // 2
                                nc.tensor.matmul(
                                    ps2[64 * half:64 * half + 64, j % 2, 0:N2],
                                    w2s[32 * j:32 * j + 32, it, :],
                                    x2p[32 * j:32 * j + 32, kh % 2, kw % 2,
                                        kh // 2:kh // 2 + 14,
                                        kw // 2:kw // 2 + 14],
                                    start=(it == 0), stop=(it == 15),
                                    tile_position=(32 * j, 64 * half),
                                )
                        x3A = x3s.rearrange("p n h w r s -> p n r h s w")
                        for k in range(2):
                            ps2k = ps2[:, k, 0:N2].rearrange(
                                "p (o w) -> p o w", w=14)
                            for h in range(2):
                                nc.scalar.activation(
                                    x3A[:, 2 * g + k, :, h, :, :],
                                    ps2k[:, h:14:2, :],
                                    AF.Relu, bias=b2s[:, :])

                # ---- conv3: 16 taps K=64, pairs batched 8 per matmul ----
                if "conv3" in phases:
                    x4q = x4s.rearrange("p (q i l) x -> p q i l x", q=8, i=4, l=4)
                    for q in range(8):
                        ps3 = pspool.tile([128, 2, 512], F32, tag="ps")
                        for it, (kh, kw) in enumerate(TAPS):
                            for h in range(2):
                                nc.tensor.matmul(
                                    ps3[:, h, 0:8 * N3],
                                    w3s[64 * h:64 * h + 64, it, :],
                                    x3s[64 * h:64 * h + 64, 8 * q:8 * q + 8,
                                        kh % 2, kw % 2,
                                        kh // 2:kh // 2 + 6,
                                        kw // 2:kw // 2 + 6],
                                    start=(it == 0), stop=(it == 15),
                                    tile_position=(64 * h, 0),
                                )
                        for h in range(2):
                            nc.scalar.activation(
                                x4q[:, q, :, 2 * h:2 * h + 2, :],
                                ps3[:, h, 0:8 * N3].rearrange(
                                    "p (n x) -> p n x", x=N3),
                                AF.Relu, bias=b3s[:, :])

                # ---- conv4: 16 taps K=128, all frames batched ----
                if "conv4" in phases:
                    x4v = x4s.rearrange("p f (r s) -> p f r s", r=O3)
                    ps4 = pspool.tile([128, 2, 512], F32, tag="ps")
                    for it, (kh, kw) in enumerate(TAPS):
                        for mh in range(2):
                            nc.tensor.matmul(
                                ps4[:, mh, :],
                                w4s[:, it, 128 * mh:128 * mh + 128],
                                x4v[:, :, kh:kh + 3:2, kw:kw + 3:2],
                                start=(it == 0), stop=(it == 15),
                            )
                    ps4v = ps4.rearrange("p m (f x) -> p m f x", x=N4)
                    for p in range(4):
                        for mh in range(2):
                            nc.scalar.activation(encs[:, 2 * p + mh, :],
                                                 ps4v[:, mh, :, p],
                                                 AF.Relu, bias=b4s[:, mh:mh + 1])

                # ---- x-projection for LSTM layer 1, batched over (b,t) ----
                if "xw1" in phases:
                    for m in range(16):
                        psx = psspool.tile([128, F], F32, tag="pss")
                        for kt in range(8):
                            nc.tensor.matmul(psx[:, :],
                                             wx1s[:, kt, 128 * m:128 * m + 128],
                                             encs[:, kt, :],
                                             start=(kt == 0), stop=False)
                        nc.tensor.matmul(psx[:, :],
                                         wacts[:, 128 * m:128 * m + 128],
                                         acts[:, :],
                                         start=False, stop=True)
                        nc.scalar.activation(xw1s[:, m, :], psx[:, :],
                                             AF.Identity, bias=bl1s[:, m:m + 1])

                # ---- LSTM recurrence, 64 steps, batch=2 per core ----
                if "lstm" in phases:
                    h1 = st.tile([128, 8], BF16, tag="h1z")
                    c1 = st.tile([128, 8], F32, tag="c1z")
                    h2 = st.tile([128, 4], BF16, tag="h2z")
                    c2 = st.tile([128, 4], F32, tag="c2z")
                    nc.any.memset(h1[:, :], 0.0)
                    nc.any.memset(c1[:, :], 0.0)
                    nc.any.memset(h2[:, :], 0.0)
                    nc.any.memset(c2[:, :], 0.0)
                    xw1v = xw1s.rearrange("p m (b t) -> p m b t", b=BP)
                    hv = h2hist.rearrange("p k (b t) -> p k b t", b=BP)

                    for t in range(T):
                        pg1 = psspool.tile([128, 32], F32, tag="pss")
                        for m in range(16):
                            for kt in range(4):
                                nc.tensor.matmul(pg1[:, 2 * m:2 * m + 2],
                                                 wh1s[:, kt, 128 * m:128 * m + 128],
                                                 h1[:, 2 * kt:2 * kt + 2],
                                                 start=(kt == 0), stop=(kt == 3))
                        g1 = sc.tile([128, 32], F32, tag="g1")
                        nc.vector.tensor_add(g1[:, :], pg1[:, :], xw1v[:, :, :, t])
                        nc.scalar.activation(g1[:, 0:8], g1[:, 0:8], AF.Sigmoid)
                        nc.scalar.activation(g1[:, 8:16], g1[:, 8:16], AF.Tanh)
                        nc.scalar.activation(g1[:, 16:32], g1[:, 16:32], AF.Sigmoid)
                        t1 = sc.tile([128, 8], F32, tag="t1")
                        nc.vector.tensor_mul(t1[:, :], g1[:, 16:24], c1[:, :])
                        t2 = sc.tile([128, 8], F32, tag="t2")
                        nc.vector.tensor_mul(t2[:, :], g1[:, 0:8], g1[:, 8:16])
                        c1 = st.tile([128, 8], F32, tag="c1")
                        nc.vector.tensor_add(c1[:, :], t1[:, :], t2[:, :])
                        th1 = sc.tile([128, 8], F32, tag="th1")
                        nc.scalar.activation(th1[:, :], c1[:, :], AF.Tanh)
                        h1 = st.tile([128, 8], BF16, tag="h1")
                        nc.vector.tensor_mul(h1[:, :], g1[:, 24:32], th1[:, :])

                        pg2 = psspool.tile([128, 16], F32, tag="pss")
                        for m in range(8):
                            for kt in range(6):
                                rhs = (h1[:, 2 * kt:2 * kt + 2] if kt < 4
                                       else h2[:, 2 * (kt - 4):2 * (kt - 4) + 2])
                                nc.tensor.matmul(pg2[:, 2 * m:2 * m + 2],
                                                 wl2s[:, kt, 128 * m:128 * m + 128],
                                                 rhs,
                                                 start=(kt == 0), stop=(kt == 5))
                        g2 = sc.tile([128, 16], F32, tag="g2")
                        nc.vector.tensor_add(g2[:, :], pg2[:, :], bl2s[:, :])
                        nc.scalar.activation(g2[:, 0:4], g2[:, 0:4], AF.Sigmoid)
                        nc.scalar.activation(g2[:, 4:8], g2[:, 4:8], AF.Tanh)
                        nc.scalar.activation(g2[:, 8:16], g2[:, 8:16], AF.Sigmoid)
                        t3 = sc.tile([128, 4], F32, tag="t3")
                        nc.vector.tensor_mul(t3[:, :], g2[:, 8:12], c2[:, :])
                        t4 = sc.tile([128, 4], F32, tag="t4")
                        nc.vector.tensor_mul(t4[:, :], g2[:, 0:4], g2[:, 4:8])
                        c2 = st.tile([128, 4], F32, tag="c2")
                        nc.vector.tensor_add(c2[:, :], t3[:, :], t4[:, :])
                        th2 = sc.tile([128, 4], F32, tag="th2")
                        nc.scalar.activation(th2[:, :], c2[:, :], AF.Tanh)
                        h2 = st.tile([128, 4], BF16, tag="h2")
                        nc.vector.tensor_mul(h2[:, :], g2[:, 12:16], th2[:, :])
                        nc.vector.tensor_copy(
                            hv[:, :, :, t],
                            h2[:, :].rearrange("p (k b) -> p k b", k=2))

                # ---- dense head, batched over (b,t) ----
                if "head" in phases:
                    pd1 = psspool.tile([128, F], F32, tag="pss")
                    for kt in range(2):
                        nc.tensor.matmul(pd1[:, :], wd1s[:, kt, :],
                                         h2hist[:, kt, :],
                                         start=(kt == 0), stop=(kt == 1))
                    d1t = sc.tile([128, F], BF16, tag="d1t")
                    nc.scalar.activation(d1t[:, :], pd1[:, :], AF.Relu,
                                         bias=bd1s[:, :])
                    pd2 = psspool.tile([1, F], F32, tag="pss")
                    nc.tensor.matmul(pd2[:, :], wd2s[:, :], d1t[:, :],
                                     start=True, stop=True)
                    osb = sc.tile([1, F], F32, tag="osb")
                    nc.scalar.activation(osb[:, :], pd2[:, :], AF.Identity,
                                         bias=bd2s[:, :])
                else:
                    osb = sc.tile([1, F], F32, tag="osb")
                    nc.any.memset(osb[:, :], 0.0)
            nc.sync.dma_start(out=outp[:, :], in_=osb[:, :])

    nc.finalize()
    return nc


def _prep_host(inputs):
    bf = ml_dtypes.bfloat16
    frames = np.ascontiguousarray(inputs["frames"], dtype=np.float32)
    actions = np.ascontiguousarray(inputs["actions"], dtype=np.float32)

    shared = {}
    w1r = np.asarray(inputs["w1"], np.float32).reshape(2, 2, 2, 2, 3, 32)
    # rows (a, di, dj, c), second dim b: matches device ict2 layout
    shared["w1b"] = np.ascontiguousarray(
        w1r.transpose(0, 1, 3, 4, 2, 5).reshape(24, 2, 32)).astype(bf)
    shared["b1p"] = np.tile(np.asarray(inputs["b1"], np.float32), 4)[:, None].copy()
    shared["w2p"] = np.ascontiguousarray(
        np.asarray(inputs["w2"], np.float32).reshape(16, 32, 64).transpose(1, 0, 2)
    ).astype(bf)
    shared["b2p"] = np.tile(np.asarray(inputs["b2"], np.float32), 2)[:, None].copy()
    shared["b3p"] = np.asarray(inputs["b3"], np.float32)[:, None].copy()
    shared["b4p"] = np.ascontiguousarray(
        np.asarray(inputs["b4"], np.float32).reshape(2, 128).T)
    kl1 = np.asarray(inputs["k_l1"], np.float32)
    shared["wactp"] = np.ascontiguousarray(kl1[1024:1026]).astype(bf)
    bl1 = np.asarray(inputs["b_l1"], np.float32).copy()
    bl1[1024:1536] += 1.0          # fold forget bias
    shared["bl1p"] = np.ascontiguousarray(bl1.reshape(16, 128).T)
    bl2 = np.asarray(inputs["b_l2"], np.float32).copy()
    bl2[512:768] += 1.0
    shared["bl2p"] = np.ascontiguousarray(
        np.repeat(bl2.reshape(8, 128).T, 2, axis=1))
    shared["bd1p"] = np.asarray(inputs["b_d1"], np.float32)[:, None].copy()
    shared["wd2p"] = np.asarray(inputs["w_d2"], np.float32).astype(bf).copy()
    shared["bd2p"] = np.asarray(inputs["b_d2"], np.float32).reshape(1, 1).copy()

    # ---- packed big-weight buffer [128, CPACK], sharded 16 rows/core ----
    wx1 = kl1[0:1024].reshape(8, 128, 2048).transpose(1, 0, 2).reshape(128, 16384)
    wh1 = kl1[1026:1538].reshape(4, 128, 2048).transpose(1, 0, 2).reshape(128, 8192)
    kl2 = np.asarray(inputs["k_l2"], np.float32)
    wl2 = kl2.reshape(6, 128, 1024).transpose(1, 0, 2).reshape(128, 6144)
    w4 = np.asarray(inputs["w4"], np.float32).reshape(16, 128, 256)
    w4 = w4.transpose(1, 0, 2).reshape(128, 4096)
    wd1 = np.asarray(inputs["w_d1"], np.float32).reshape(2, 128, 128)
    wd1 = wd1.transpose(1, 0, 2).reshape(128, 256)
    w3 = np.asarray(inputs["w3"], np.float32).reshape(16, 64, 128)
    w3 = w3.transpose(1, 0, 2).reshape(64, 2048)
    w3pack = np.concatenate([w3[:, 0:1024], w3[:, 1024:2048]], axis=0)
    wbig = np.concatenate([wx1, wh1, wl2, w4, wd1, w3pack], axis=1).astype(bf)
    assert wbig.shape == (128, CPACK)

    in_maps = []
    for c in range(NCORES):
        fr = frames[BP * c:BP * c + BP].reshape(F, H, W, C)
        z = fr.reshape(F, 32, 2, 32, 2, 3).transpose(2, 4, 5, 0, 1, 3)
        zd = np.ascontiguousarray(z.reshape(12, F, 32, 32)).astype(bf)
        actT = np.ascontiguousarray(
            actions[BP * c:BP * c + BP].reshape(F, 2).T).astype(bf)
        m = {"zd": zd, "actT": actT,
             "wsh": np.ascontiguousarray(wbig[16 * c:16 * c + 16]),
             "wbig": wbig}
        m.update(shared)
        in_maps.append(m)
    return in_maps


def kernel(**inputs):
    from concourse.bass_utils import run_bass_kernel_spmd
    if "nc" not in _CACHE:
        _CACHE["nc"] = _build_graph()
    nc = _CACHE["nc"]
    in_maps = _prep_host(inputs)
    res = run_bass_kernel_spmd(nc, in_maps, core_ids=list(range(NCORES)),
                               trace=False)
    outs = [res.results[c]["out"].reshape(BP, T, 1) for c in range(NCORES)]
    return np.concatenate(outs, axis=0)
